# revision 1
# baseline (speedup 1.0000x reference)
"""BitTransformerLayer on 8 Trainium2 NeuronCores.

Data-parallel over batch: each core runs one batch element [S=1024, D=1024]
through the full layer. No collectives.

Per-core dataflow (all matmuls contract over the SBUF partition dim):
  - x loaded token-major [tok, d]; RMSNorm per token (free-dim stats).
  - xn transposed to feature-major via PE transpose -> xnT (fp32r).
  - QKV: Q^T,K^T produced feature-major (weights stationary); V token-major
    with a ones-column appended (softmax denominator rides the AV matmul).
  - Attention per head: S^T = K^T.T @ Q^T (fp32r), exp on ScalarE
    (scale=1/sqrt(dh) folded), O~^T = [V|1].T @ E^T. Row 64 of O~ = denom.
  - Unnormalized O^T staged to DRAM; denominators batch-reciprocaled and
    broadcast back (DRAM step-0 partition AP); normalize -> O_norm^T (fp32r).
  - out_proj token-major (O_norm^T stationary), residual add.
  - RMSNorm2 + act_quant (exact: abs-max, s=127/m via divide, magic-number
    round-half-even, int8 values stored as bf16 -> exact bf16 matmuls).
  - FFN with host-ternarized weights in bf16 ({-1,0,1} exact); integer
    accumulation in PSUM is exact; scales folded into the Gelu activation
    and the fused (psum*scale + residual) epilogue.

fp32r (=fp32 rounded to 11 mantissa bits on the PE) is used for all
non-integer matmuls: ~bf16 speed, ~14x lower error than bf16.

SBUF: big resident tensors live in a hand-drawn arena (alloc_sbuf_tensor_at,
regions aliased across stages; Tile's OverlapTracker fences reuse). Small
rotating buffers use strictly-LIFO tile pools above the arena.
"""
import sys

for _p in ("/opt/trn_rl_repo", "/opt/pypackages"):
    if _p not in sys.path:
        sys.path.append(_p)

import numpy as np
import concourse.bass as bass
import concourse.tile as tile
from concourse import bacc, mybir
from concourse.bass_utils import run_bass_kernel_spmd
from concourse.masks import make_identity

FP32 = mybir.dt.float32
FP32R = mybir.dt.float32r
BF16 = mybir.dt.bfloat16

B, S, D, H, FF = 8, 1024, 1024, 16, 4096
DH = D // H          # 64
T = S // 128         # token tiles
C = D // 128         # d chunks
FC = FF // 128       # ff chunks
FH = FF // 512       # ff halves (512-wide)
QH = S // 512        # query halves
EPS = 1e-6
MAGIC = float(1.5 * 2 ** 23)

Act = mybir.ActivationFunctionType
Alu = mybir.AluOpType

_last_results = None  # test harness can inspect exec_time_ns etc.


def _build(w1s: float, w2s: float, flags: dict):
    nc = bacc.Bacc()

    x_d = nc.declare_dram_parameter("x", [S, D], FP32, isOutput=False)
    wqkvT_d = nc.declare_dram_parameter("wqkvT", [D, 3 * D], FP32R, isOutput=False)
    woT_d = nc.declare_dram_parameter("woT", [D, D], FP32R, isOutput=False)
    w1qT_d = nc.declare_dram_parameter("w1qT", [D, FF], BF16, isOutput=False)
    w2qT_d = nc.declare_dram_parameter("w2qT", [FF, D], BF16, isOutput=False)
    extras = {}
    for nm, shp, fl in (("bqkv", [3 * D], "bqkv"), ("bo", [D], "bo"),
                        ("b1", [FF], "b1"), ("b2", [D], "b2"), ("n2w", [D], "n2w")):
        if flags[fl]:
            extras[nm] = nc.declare_dram_parameter(nm, shp, FP32, isOutput=False)
    out_d = nc.declare_dram_parameter("out", [S, D], FP32, isOutput=True)

    den_scr = nc.dram_tensor("den_scr", [QH, 16, 512], FP32)
    oc_scr = nc.dram_tensor("oc_scr", [D, S], FP32)     # unnormalized O^T
    hq_scr = nc.dram_tensor("hq_scr", [S, FF], BF16)

    # ---- hand-drawn SBUF arena (per-partition byte offsets) ----
    A0 = 16512
    R0 = A0                    # 32KB: x/xn (A-C) then x1 (F-I)
    R1 = A0 + 32 * 1024        # 32KB: xnT (C-D) then ocat_n (E-F)
    R2 = A0 + 64 * 1024        # 64KB: qk (D-E) then w1sb (H) then hqT (I)
    R3 = A0 + 128 * 1024       # 34KB: vaug (D-E) then h double-buffer (H)
    ARENA_END = A0 + 162 * 1024
    nc.sbuf_base = ARENA_END   # rotating tile pools live above the arena

    man = nc.alloc_sbuf_tensor_at
    x_sb = [man(f"x_{t}", [128, D], FP32, offset=R0 + t * 4096) for t in range(T)]
    x1 = [man(f"x1_{t}", [128, D], FP32, offset=R0 + t * 4096) for t in range(T)]
    xnT = [man(f"xnT{c}", [128, S], FP32R, offset=R1 + c * 4096) for c in range(C)]
    ocat_n = [man(f"ocn{c}", [128, S], FP32R, offset=R1 + c * 4096) for c in range(C)]
    qk = [man(f"qk{f}", [128, S], FP32R, offset=R2 + f * 4096) for f in range(16)]
    w1sb = [man(f"w1_{c}", [128, FF], BF16, offset=R2 + c * 8192) for c in range(C)]
    hqT = [man(f"hqT{fc}", [128, S], BF16, offset=R2 + fc * 2048) for fc in range(FC)]
    vaug = [man(f"va{t}", [128, H, DH + 1], FP32R, offset=R3 + t * 4160)
            for t in range(T)]
    h_db = [man(f"h_{i}", [128, FF], FP32, offset=R3 + i * 16384) for i in range(2)]

    def bcast_row(dram_ap, lo, n, width, pool, tag, parts=128):
        t_ = pool.tile([parts, width], FP32, tag=tag, name=tag)
        ap = bass.AP(tensor=dram_ap.tensor, offset=dram_ap.offset + lo,
                     ap=[[width, n], [0, parts // n], [1, width]])
        nc.sync.dma_start(out=t_, in_=ap)
        return t_

    with tile.TileContext(nc) as tc:
        small_cm = tc.tile_pool(name="small", bufs=1)
        small = small_cm.__enter__()

        eps_t = small.tile([128, 1], FP32, tag="eps", name="eps")
        nc.vector.memset(eps_t, EPS)
        c127 = small.tile([128, 1], FP32, tag="c127", name="c127")
        nc.vector.memset(c127, 127.0)
        ident = small.tile([128, 128], FP32, tag="ident", name="ident")
        make_identity(nc, ident)
        ident_bf = small.tile([128, 128], BF16, tag="identbf", name="identbf")
        make_identity(nc, ident_bf)

        den16 = [small.tile([16, 512], FP32, tag=f"den{qh}", name=f"den{qh}")
                 for qh in range(QH)]
        sfac = [small.tile([128, 1], FP32, tag=f"sfac{t}", name=f"sfac{t}")
                for t in range(T)]
        gfac = [small.tile([128, 1], FP32, tag=f"gfac{t}", name=f"gfac{t}")
                for t in range(T)]

        # ============ Stage A/B/C: load x, RMSNorm1, transpose ============
        pscr_cm = tc.tile_pool(name="pscr", bufs=2)
        pscr = pscr_cm.__enter__()
        psA_cm = tc.tile_pool(name="psA", bufs=2, space="PSUM")
        psA = psA_cm.__enter__()

        for t in range(T):
            x_t = x_sb[t]
            nc.sync.dma_start(out=x_t[:], in_=x_d[t * 128:(t + 1) * 128, :])
            scr = pscr.tile([128, D], FP32, tag="sqscr", name="sqscr")
            ssq = pscr.tile([128, 1], FP32, tag="ssq", name="ssq")
            nc.scalar.activation(scr, x_t[:], Act.Square, accum_out=ssq)
            rstd = pscr.tile([128, 1], FP32, tag="rstd", name="rstd")
            nc.scalar.activation(rstd, ssq, Act.Sqrt, bias=eps_t, scale=1.0 / D)
            nc.vector.reciprocal(rstd, rstd)
            nc.vector.tensor_scalar_mul(out=x_t[:], in0=x_t[:], scalar1=rstd)
            for c in range(C):
                tp = psA.tile([128, 128], FP32, tag="tp", name="tp")
                nc.tensor.transpose(tp, x_t[:, c * 128:(c + 1) * 128], ident)
                nc.vector.tensor_copy(out=xnT[c][:, t * 128:(t + 1) * 128], in_=tp)
        psA_cm.__exit__(None, None, None)
        pscr_cm.__exit__(None, None, None)

        # ============ Stage D: QKV projections ============
        pwq_cm = tc.tile_pool(name="pwq", bufs=4)
        pwq = pwq_cm.__enter__()
        psD_cm = tc.tile_pool(name="psD", bufs=1, space="PSUM")
        psD = psD_cm.__enter__()

        def _qk_epilogue(f, ps_pair):
            if flags["bqkv"]:
                bq_f = small.tile([128, 1], FP32, tag=f"bq{f}", name=f"bq{f}")
                nc.sync.dma_start(
                    out=bq_f,
                    in_=extras["bqkv"][f * 128:(f + 1) * 128].rearrange(
                        "(p o) -> p o", o=1))
                for n in range(QH):
                    tmpb = pwq.tile([128, 512], FP32, tag="tmpb", name="tmpb")
                    nc.vector.tensor_scalar_add(out=tmpb, in0=ps_pair[n],
                                                scalar1=bq_f)
                    nc.vector.tensor_copy(out=qk[f][:, n * 512:(n + 1) * 512],
                                          in_=tmpb)
            else:
                for n in range(QH):
                    nc.vector.tensor_copy(out=qk[f][:, n * 512:(n + 1) * 512],
                                          in_=ps_pair[n])

        for fg in range(4):  # 16 f-tiles (Q: 0..7, K: 8..15) in groups of 4
            qk_ps = [[psD.tile([128, 512], FP32, tag=f"qkps{fi}_{n}",
                               name=f"qkps{fi}_{n}") for n in range(QH)]
                     for fi in range(4)]
            for c in range(C):
                wq4 = pwq.tile([128, 512], FP32R, tag="wq4", name="wq4")
                nc.sync.dma_start(
                    out=wq4,
                    in_=wqkvT_d[c * 128:(c + 1) * 128, fg * 512:(fg + 1) * 512])
                for fi in range(4):
                    for n in range(QH):
                        nc.tensor.matmul(qk_ps[fi][n],
                                         lhsT=wq4[:, fi * 128:(fi + 1) * 128],
                                         rhs=xnT[c][:, n * 512:(n + 1) * 512],
                                         start=(c == 0), stop=(c == C - 1))
            for fi in range(4):
                _qk_epilogue(fg * 4 + fi, qk_ps[fi])
        psD_cm.__exit__(None, None, None)

        psV_cm = tc.tile_pool(name="psV", bufs=1, space="PSUM")
        psV = psV_cm.__enter__()
        ones16 = small.tile([128, H, 1], FP32, tag="ones16", name="ones16")
        nc.vector.memset(ones16, 1.0)
        for t in range(T):
            nc.vector.tensor_copy(out=vaug[t][:, :, DH:DH + 1], in_=ones16)
        for vh in range(2):
            v_ps = [psV.tile([128, 512], FP32, tag=f"vps{t}", name=f"vps{t}")
                    for t in range(T)]
            for c in range(C):
                wv = pwq.tile([128, 512], FP32R, tag="wv", name="wv")
                nc.sync.dma_start(
                    out=wv,
                    in_=wqkvT_d[c * 128:(c + 1) * 128,
                                2 * D + vh * 512: 2 * D + (vh + 1) * 512])
                for t in range(T):
                    nc.tensor.matmul(v_ps[t], lhsT=xnT[c][:, t * 128:(t + 1) * 128],
                                     rhs=wv, start=(c == 0), stop=(c == C - 1))
            for t in range(T):
                src = v_ps[t].rearrange("p (hh dd) -> p hh dd", dd=DH)
                dst = vaug[t][:, vh * 8:(vh + 1) * 8, 0:DH]
                if flags["bqkv"]:
                    bvb = bcast_row(extras["bqkv"][:], 2 * D + vh * 512, 1, 512,
                                    pwq, "bvb")
                    tmpv = pwq.tile([128, 512], FP32, tag="tmpv", name="tmpv")
                    nc.vector.tensor_add(
                        out=tmpv.rearrange("p (hh dd) -> p hh dd", dd=DH),
                        in0=src,
                        in1=bvb.rearrange("p (hh dd) -> p hh dd", dd=DH))
                    nc.vector.tensor_copy(
                        out=dst,
                        in_=tmpv.rearrange("p (hh dd) -> p hh dd", dd=DH))
                else:
                    nc.vector.tensor_copy(out=dst, in_=src)
        psV_cm.__exit__(None, None, None)
        pwq_cm.__exit__(None, None, None)

        # ============ Stage E: attention ============
        pet_cm = tc.tile_pool(name="pet", bufs=3)
        pet = pet_cm.__enter__()
        postg_cm = tc.tile_pool(name="postg", bufs=3)
        postg = postg_cm.__enter__()
        psS_cm = tc.tile_pool(name="psS", bufs=3, space="PSUM")
        psS = psS_cm.__enter__()
        psO_cm = tc.tile_pool(name="psO", bufs=1, space="PSUM")
        psO = psO_cm.__enter__()

        for h in range(H):
            ft = h // 2
            bq = (h % 2) * 64
            o_pss = [psO.tile([DH + 1, 512], FP32, tag=f"ops{qh}", name=f"ops{qh}")
                     for qh in range(QH)]
            for kt in range(T):
                s_ps = psS.tile([128, S], FP32, tag="sps", name="sps")
                for qh in range(QH):
                    nc.tensor.matmul(
                        s_ps[:, qh * 512:(qh + 1) * 512],
                        lhsT=qk[8 + ft][bq:bq + 64, kt * 128:(kt + 1) * 128],
                        rhs=qk[ft][bq:bq + 64, qh * 512:(qh + 1) * 512],
                        start=True, stop=True)
                et = pet.tile([128, S], FP32R, tag="et", name="et")
                nc.scalar.activation(et, s_ps, Act.Exp,
                                     scale=float(1.0 / np.sqrt(DH)))
                for qh in range(QH):
                    nc.tensor.matmul(o_pss[qh], lhsT=vaug[kt][:, h, :],
                                     rhs=et[:, qh * 512:(qh + 1) * 512],
                                     start=(kt == 0), stop=(kt == T - 1))
            for qh in range(QH):
                stg = postg.tile([64, 512], FP32, tag="stg", name="stg")
                nc.vector.tensor_copy(out=stg, in_=o_pss[qh][0:DH, :])
                nc.gpsimd.dma_start(
                    out=oc_scr[h * 64:(h + 1) * 64, qh * 512:(qh + 1) * 512],
                    in_=stg)
                std = postg.tile([1, 512], FP32, tag="std", name="std")
                nc.vector.tensor_copy(out=std, in_=o_pss[qh][DH:DH + 1, :])
                nc.gpsimd.dma_start(out=den_scr[qh, h], in_=std)
        psO_cm.__exit__(None, None, None)
        psS_cm.__exit__(None, None, None)
        postg_cm.__exit__(None, None, None)
        pet_cm.__exit__(None, None, None)

        for qh in range(QH):
            nc.sync.dma_start(out=den16[qh], in_=den_scr[qh])
            nc.vector.reciprocal(den16[qh], den16[qh])
            nc.sync.dma_start(out=den_scr[qh], in_=den16[qh])

        # ============ Stage F: normalize O^T per chunk, interleaved with
        # the out_proj matmuls (normalize(c+1) overlaps MMs(c) on the PE)
        prl_cm = tc.tile_pool(name="prl", bufs=3)
        prl = prl_cm.__enter__()
        pdb_cm = tc.tile_pool(name="pdb", bufs=2)
        pdb = pdb_cm.__enter__()
        pwo_cm = tc.tile_pool(name="pwo", bufs=3)
        pwo = pwo_cm.__enter__()
        pxr_cm = tc.tile_pool(name="pxr", bufs=3)
        pxr = pxr_cm.__enter__()
        psF_cm = tc.tile_pool(name="psF", bufs=1, space="PSUM")
        psF = psF_cm.__enter__()
        dv = den_scr[:]

        def _normalize_chunk(c):
            for qh in range(QH):
                rl = prl.tile([128, 512], FP32, tag="rl", name="rl")
                nc.scalar.dma_start(
                    out=rl,
                    in_=oc_scr[c * 128:(c + 1) * 128, qh * 512:(qh + 1) * 512])
                db = pdb.tile([128, 512], FP32, tag="db", name="db")
                ap = bass.AP(tensor=dv.tensor, offset=(qh * 16 + 2 * c) * 512,
                             ap=[[512, 2], [0, 64], [1, 512]])
                nc.sync.dma_start(out=db, in_=ap)
                tmpn = pdb.tile([128, 512], FP32, tag="tmpn", name="tmpn")
                nc.vector.tensor_mul(out=tmpn, in0=rl, in1=db)
                nc.vector.tensor_copy(out=ocat_n[c][:, qh * 512:(qh + 1) * 512],
                                      in_=tmpn)

        for oh in range(2):
            x1_ps = [psF.tile([128, 512], FP32, tag=f"x1ps{t}", name=f"x1ps{t}")
                     for t in range(T)]
            for c in range(C):
                if oh == 0:
                    _normalize_chunk(c)
                wo = pwo.tile([128, 512], FP32R, tag="wo", name="wo")
                nc.sync.dma_start(
                    out=wo,
                    in_=woT_d[c * 128:(c + 1) * 128, oh * 512:(oh + 1) * 512])
                for t in range(T):
                    nc.tensor.matmul(x1_ps[t],
                                     lhsT=ocat_n[c][:, t * 128:(t + 1) * 128],
                                     rhs=wo, start=(c == 0), stop=(c == C - 1))
            bob = None
            if flags["bo"]:
                bob = bcast_row(extras["bo"][:], oh * 512, 1, 512, pwo, "bob")
            for t in range(T):
                xr = pxr.tile([128, 512], FP32, tag="xr", name="xr")
                nc.gpsimd.dma_start(
                    out=xr, in_=x_d[t * 128:(t + 1) * 128, oh * 512:(oh + 1) * 512])
                dst = x1[t][:, oh * 512:(oh + 1) * 512]
                nc.vector.tensor_add(out=dst, in0=x1_ps[t], in1=xr)
                if bob is not None:
                    nc.vector.tensor_add(out=dst, in0=dst, in1=bob)
        psF_cm.__exit__(None, None, None)
        pxr_cm.__exit__(None, None, None)
        pwo_cm.__exit__(None, None, None)
        pdb_cm.__exit__(None, None, None)
        prl_cm.__exit__(None, None, None)

        # ============ Stage G: RMSNorm2 + act_quant + transpose ============
        pyqT_cm = tc.tile_pool(name="pyqT", bufs=1)
        pyqT = pyqT_cm.__enter__()
        yqT = [pyqT.tile([128, S], BF16, tag=f"yqT{c}", name=f"yqT{c}")
               for c in range(C)]
        pg_cm = tc.tile_pool(name="pg", bufs=2)
        pg = pg_cm.__enter__()
        pyq_cm = tc.tile_pool(name="pyq", bufs=3)
        pyq = pyq_cm.__enter__()
        psG_cm = tc.tile_pool(name="psG", bufs=4, space="PSUM")
        psG = psG_cm.__enter__()

        n2wb = None
        if flags["n2w"]:
            n2wb = bcast_row(extras["n2w"][:], 0, 1, D, small, "n2wb")

        for t in range(T):
            scr = pg.tile([128, D], FP32, tag="sqscr2", name="sqscr2")
            ssq = pg.tile([128, 1], FP32, tag="ssq2", name="ssq2")
            nc.scalar.activation(scr, x1[t][:], Act.Square, accum_out=ssq)
            rstd = pg.tile([128, 1], FP32, tag="rstd2", name="rstd2")
            nc.scalar.activation(rstd, ssq, Act.Sqrt, bias=eps_t, scale=1.0 / D)
            nc.vector.reciprocal(rstd, rstd)
            y_t = pg.tile([128, D], FP32, tag="yt", name="yt")
            nc.vector.tensor_scalar_mul(out=y_t, in0=x1[t][:], scalar1=rstd)
            if n2wb is not None:
                nc.vector.tensor_mul(out=y_t, in0=y_t, in1=n2wb)
            m_t = pg.tile([128, 1], FP32, tag="mt", name="mt")
            nc.vector.tensor_reduce(out=m_t, in_=y_t, axis=mybir.AxisListType.X,
                                    op=Alu.max, apply_absolute_value=True)
            nc.vector.tensor_scalar_max(out=m_t, in0=m_t, scalar1=1e-5)
            s_t = pg.tile([128, 1], FP32, tag="st", name="st")
            nc.vector.reciprocal(s_t, m_t)
            nc.vector.tensor_scalar_mul(out=s_t, in0=s_t, scalar1=127.0)
            nc.vector.tensor_scalar_mul(out=sfac[t], in0=m_t,
                                        scalar1=float(w1s / 127.0))
            nc.vector.tensor_scalar(out=y_t, in0=y_t, scalar1=s_t, scalar2=MAGIC,
                                    op0=Alu.mult, op1=Alu.add)
            yq_t = pyq.tile([128, D], BF16, tag="yq", name="yq")
            nc.vector.tensor_scalar(out=yq_t, in0=y_t, scalar1=-MAGIC,
                                    scalar2=None, op0=Alu.add)
            for c in range(C):
                tpq = psG.tile([128, 128], BF16, tag="tpq", name="tpq")
                nc.tensor.transpose(tpq, yq_t[:, c * 128:(c + 1) * 128], ident_bf)
                nc.vector.tensor_copy(out=yqT[c][:, t * 128:(t + 1) * 128],
                                      in_=tpq)
        psG_cm.__exit__(None, None, None)
        pyq_cm.__exit__(None, None, None)
        pg_cm.__exit__(None, None, None)

        # ============ Stage H: FFN1 + gelu + act_quant2 ============
        for c in range(C):
            nc.sync.dma_start(out=w1sb[c][:], in_=w1qT_d[c * 128:(c + 1) * 128, :])
        b1b = []
        phq_cm = tc.tile_pool(name="phq", bufs=2)
        phq = phq_cm.__enter__()
        pg2_cm = tc.tile_pool(name="pg2", bufs=2)
        pg2 = pg2_cm.__enter__()
        if flags["b1"]:
            for fh in range(FH):
                b1b.append(bcast_row(extras["b1"][:], fh * 512, 1, 512,
                                     pg2, f"b1b{fh}"))
        psH_cm = tc.tile_pool(name="psH", bufs=1, space="PSUM")
        psH = psH_cm.__enter__()

        for t in range(T):
            h_ps = [psH.tile([128, 512], FP32, tag=f"hps{fh}", name=f"hps{fh}")
                    for fh in range(FH)]
            for c in range(C):
                for fh in range(FH):
                    nc.tensor.matmul(h_ps[fh],
                                     lhsT=yqT[c][:, t * 128:(t + 1) * 128],
                                     rhs=w1sb[c][:, fh * 512:(fh + 1) * 512],
                                     start=(c == 0), stop=(c == C - 1))
            h_t = h_db[t % 2]
            for fh in range(FH):
                hslice = h_t[:, fh * 512:(fh + 1) * 512]
                if flags["b1"]:
                    tmp = pg2.tile([128, 512], FP32, tag="b1tmp", name="b1tmp")
                    nc.vector.tensor_scalar_mul(out=tmp, in0=h_ps[fh],
                                                scalar1=sfac[t])
                    nc.vector.tensor_add(out=tmp, in0=tmp, in1=b1b[fh])
                    nc.scalar.activation(hslice, tmp, Act.Gelu)
                else:
                    nc.scalar.activation(hslice, h_ps[fh], Act.Gelu,
                                         scale=sfac[t])
            m2 = pg2.tile([128, 1], FP32, tag="m2", name="m2")
            nc.vector.tensor_reduce(out=m2, in_=h_t[:], axis=mybir.AxisListType.X,
                                    op=Alu.max, apply_absolute_value=True)
            nc.vector.tensor_scalar_max(out=m2, in0=m2, scalar1=1e-5)
            s2 = pg2.tile([128, 1], FP32, tag="s2", name="s2")
            nc.vector.reciprocal(s2, m2)
            nc.vector.tensor_scalar_mul(out=s2, in0=s2, scalar1=127.0)
            nc.vector.tensor_scalar_mul(out=gfac[t], in0=m2,
                                        scalar1=float(w2s / 127.0))
            nc.vector.tensor_scalar(out=h_t[:], in0=h_t[:], scalar1=s2,
                                    scalar2=MAGIC, op0=Alu.mult, op1=Alu.add)
            hq_t = phq.tile([128, FF], BF16, tag="hq", name="hq")
            nc.vector.tensor_scalar(out=hq_t, in0=h_t[:], scalar1=-MAGIC,
                                    scalar2=None, op0=Alu.add)
            nc.gpsimd.dma_start(out=hq_scr[t * 128:(t + 1) * 128, :], in_=hq_t)
        psH_cm.__exit__(None, None, None)
        pg2_cm.__exit__(None, None, None)
        phq_cm.__exit__(None, None, None)
        pyqT_cm.__exit__(None, None, None)

        # ============ Stage I: FFN2 + residual -> out ============
        pld_cm = tc.tile_pool(name="pld", bufs=3)
        pld = pld_cm.__enter__()
        psT_cm = tc.tile_pool(name="psT", bufs=4, space="PSUM")
        psT = psT_cm.__enter__()
        for t in range(T):
            for fg in range(FC // 8):
                ld = pld.tile([128, 1024], BF16, tag="ld", name="ld")
                nc.sync.dma_start(
                    out=ld,
                    in_=hq_scr[t * 128:(t + 1) * 128, fg * 1024:(fg + 1) * 1024])
                for fi in range(8):
                    fc = fg * 8 + fi
                    tp = psT.tile([128, 128], BF16, tag="tph", name="tph")
                    nc.tensor.transpose(tp, ld[:, fi * 128:(fi + 1) * 128],
                                        ident_bf)
                    nc.vector.tensor_copy(out=hqT[fc][:, t * 128:(t + 1) * 128],
                                          in_=tp)
        psT_cm.__exit__(None, None, None)
        pld_cm.__exit__(None, None, None)

        pw2_cm = tc.tile_pool(name="pw2", bufs=3)
        pw2 = pw2_cm.__enter__()
        pout_cm = tc.tile_pool(name="pout", bufs=3)
        pout = pout_cm.__enter__()
        psI_cm = tc.tile_pool(name="psI", bufs=1, space="PSUM")
        psI = psI_cm.__enter__()
        for oh in range(2):
            o2_ps = [psI.tile([128, 512], FP32, tag=f"o2ps{t}", name=f"o2ps{t}")
                     for t in range(T)]
            for fc in range(FC):
                w2t = pw2.tile([128, 512], BF16, tag="w2", name="w2")
                nc.sync.dma_start(
                    out=w2t,
                    in_=w2qT_d[fc * 128:(fc + 1) * 128, oh * 512:(oh + 1) * 512])
                for t in range(T):
                    nc.tensor.matmul(o2_ps[t],
                                     lhsT=hqT[fc][:, t * 128:(t + 1) * 128],
                                     rhs=w2t, start=(fc == 0), stop=(fc == FC - 1))
            b2b = None
            if flags["b2"]:
                b2b = bcast_row(extras["b2"][:], oh * 512, 1, 512, pw2, "b2b")
            for t in range(T):
                ot = pout.tile([128, 512], FP32, tag="ot", name="ot")
                nc.vector.scalar_tensor_tensor(
                    out=ot, in0=o2_ps[t], scalar=gfac[t],
                    in1=x1[t][:, oh * 512:(oh + 1) * 512],
                    op0=Alu.mult, op1=Alu.add)
                if b2b is not None:
                    nc.vector.tensor_add(out=ot, in0=ot, in1=b2b)
                nc.gpsimd.dma_start(
                    out=out_d[t * 128:(t + 1) * 128, oh * 512:(oh + 1) * 512],
                    in_=ot)
        psI_cm.__exit__(None, None, None)
        pout_cm.__exit__(None, None, None)
        pw2_cm.__exit__(None, None, None)
        small_cm.__exit__(None, None, None)

    nc.finalize()
    return nc


def kernel(**inputs):
    global _last_results
    x = np.ascontiguousarray(np.asarray(inputs["x"], dtype=np.float32))
    n1 = np.asarray(inputs["norm1_w"], dtype=np.float32)
    n2 = np.asarray(inputs["norm2_w"], dtype=np.float32)
    wqkv = np.asarray(inputs["in_proj_w"], dtype=np.float32)
    bqkv = np.asarray(inputs["in_proj_b"], dtype=np.float32)
    wo = np.asarray(inputs["out_proj_w"], dtype=np.float32)
    bo = np.asarray(inputs["out_proj_b"], dtype=np.float32)
    w1 = np.asarray(inputs["w1"], dtype=np.float32)
    b1 = np.asarray(inputs["b1"], dtype=np.float32)
    w2 = np.asarray(inputs["w2"], dtype=np.float32)
    b2 = np.asarray(inputs["b2"], dtype=np.float32)

    import ml_dtypes

    # fold norm1_w into the qkv weight (rmsnorm scale commutes; the per-d
    # norm weight multiplies the contraction dim)
    wqkvT = np.ascontiguousarray((wqkv * n1[None, :]).T.astype(np.float32))
    woT = np.ascontiguousarray(wo.T.astype(np.float32))

    def ternarize(w):
        s = np.float32(1.0) / np.clip(np.abs(w).mean(dtype=np.float32),
                                      np.float32(1e-5), None)
        q = np.clip(np.round(w * s), -1.0, 1.0).astype(np.float32)
        return q, float(np.float32(1.0) / s)

    w1q, w1s = ternarize(w1)
    w2q, w2s = ternarize(w2)
    w1qT = np.ascontiguousarray(w1q.T).astype(ml_dtypes.bfloat16)
    w2qT = np.ascontiguousarray(w2q.T).astype(ml_dtypes.bfloat16)

    flags = {
        "bqkv": bool(np.any(bqkv != 0)),
        "bo": bool(np.any(bo != 0)),
        "b1": bool(np.any(b1 != 0)),
        "b2": bool(np.any(b2 != 0)),
        "n2w": not bool(np.all(n2 == 1.0)),
    }

    nc = _build(w1s, w2s, flags)

    shared = dict(wqkvT=wqkvT, woT=woT, w1qT=w1qT, w2qT=w2qT)
    for nm, arr in (("bqkv", bqkv), ("bo", bo), ("b1", b1), ("b2", b2),
                    ("n2w", n2)):
        if flags[nm]:
            shared[nm] = arr

    in_maps = [dict(x=np.ascontiguousarray(x[b]), **shared) for b in range(B)]
    res = run_bass_kernel_spmd(nc, in_maps, list(range(B)))
    _last_results = res
    return np.stack([res.results[b]["out"] for b in range(B)]).astype(np.float32)

